# revision 4
# baseline (speedup 1.0000x reference)
# Trainium2 Bass kernel for nn_CustomKeypointLoss.
#
# reference(...) = sum over batch of:
#   sum_k |kp - gt|  +  10 * sum_{3 masks} [ quant_off + 10 * sum_k (1 - mask[b, ix, iy]) ]
# where kp = argmax-derived normalized keypoints from pred_heatmaps [B,K,512,512].
#
# Since kp in [0,1], ix=floor(kp_x) and iy=floor(kp_y) are in {0,1}: the masks are
# only read at [:, 0:2, 0:2].  All heavy lifting is the argmax over the 268MB of
# heatmaps.  Data-parallel over 8 cores (4 batch images each).
#
# Per-core device kernel:
#   view the core's heatmaps as hm[4096, 2048] (32 images x 128 chunks x 2048).
#   Stage A: 16x 2MB DMAs (two images per transfer, alternating between the two
#            HWDGE queues: sync + scalar) -> vector.reduce_max per image chunk ->
#            redmax[128, 32].  One full-data DVE scan, hidden under the DMA stream.
#   Stage B (per half of 16 images, so half 0 overlaps half 1's streaming):
#            PE-transpose redmax half [128,16] -> [16,128]; vector.max / max_index
#            give each image's global max and the FIRST 2048-chunk containing it.
#   Stage C: indirect-DMA gather of the 16 winning rows hm[img*128 + p_win, :]
#            from HBM; vector.max_index (reusing stage-B top8 maxes) gives the
#            first in-row index of the max.
#   Output: out_idx[32, 2] = (p_win, in_idx); flat argmax = p_win*2048 + in_idx.
#
# Host: reconstruct (x, y) = (flat % 512, flat // 512) and evaluate the (tiny)
# loss arithmetic in float32 exactly like the reference; sum partials over cores.

import numpy as np

B, K, H, W = 32, 8, 512, 512
N_CORES = 8
B_PER = B // N_CORES          # images per core
TILES = B_PER * K             # 32 heatmaps per core
P = 128                       # SBUF partitions
FREE = (H * W) // P           # 2048 elements per partition-row
ROWS = TILES * P              # 4096 rows in the per-core [ROWS, FREE] view
# Stream DMA sizes in images (1MB each): tapered so the pipeline starts and
# drains faster; 2MB in the middle for queue efficiency.
DMA_IMGS = [1, 1, 1, 1] + [2] * 12 + [1, 1, 1, 1]
assert sum(DMA_IMGS) == TILES
# Stage-B/C groups (image offset, count): group ends must align with DMA ends.
GROUPS = [(0, 16), (16, 8), (24, 8)]

_CACHE = {}
RUN_OPTS = {}  # test harness may set {"trace": True, ...}; harmless otherwise
LAST_RESULTS = {}  # test harness reads exec_time_ns from here


def _build():
    import concourse.bacc as bacc
    import concourse.tile as tile
    import concourse.mybir as mybir
    from concourse import bass
    from concourse.masks import make_identity

    f32 = mybir.dt.float32
    u32 = mybir.dt.uint32
    X = mybir.AxisListType.X

    nc = bacc.Bacc(
        "TRN2", target_bir_lowering=False, debug=False, enable_asserts=False
    )
    hm = nc.dram_tensor("hm", [ROWS, FREE], f32, kind="ExternalInput").ap()
    out_idx = nc.dram_tensor("out_idx", [TILES, 2], u32, kind="ExternalOutput").ap()

    with tile.TileContext(nc) as tc:
        with (
            tc.tile_pool(name="load", bufs=8) as load_pool,
            tc.tile_pool(name="stats", bufs=1) as stats,
            tc.tile_pool(name="psum", bufs=2, space="PSUM") as psum,
        ):
            ident = stats.tile([P, P], f32)
            make_identity(nc, ident[:])

            redmax = stats.tile([P, TILES], f32)

            def stage_bc(off, sz):
                """Cross-partition argmax + winning-row gather for images
                [off, off+sz)."""
                rm_t_ps = psum.tile([sz, P], f32, space="PSUM", tag="rm_t_ps")
                nc.tensor.transpose(
                    out=rm_t_ps[:],
                    in_=redmax[:, off : off + sz],
                    identity=ident[:],
                )
                rm_t = stats.tile([sz, P], f32, tag=f"rm_t{off}")
                nc.scalar.copy(rm_t[:], rm_t_ps[:])

                top8 = stats.tile([sz, 8], f32, tag=f"top8{off}")
                nc.vector.max(out=top8[:], in_=rm_t[:])
                pwin8 = stats.tile([sz, 8], u32, tag=f"pwin8{off}")
                nc.vector.max_index(out=pwin8[:], in_max=top8[:], in_values=rm_t[:])

                # global row to gather = (off + img_local)*128 + p_win
                rowidx = stats.tile([sz, 1], u32, tag=f"rowidx{off}")
                nc.gpsimd.iota(
                    rowidx[:], pattern=[[0, 1]], base=off * P, channel_multiplier=P
                )
                nc.gpsimd.tensor_tensor(
                    out=rowidx[:], in0=rowidx[:], in1=pwin8[:, 0:1],
                    op=mybir.AluOpType.add,
                )

                gath = stats.tile([sz, FREE], f32, tag=f"gath{off}")
                nc.gpsimd.indirect_dma_start(
                    out=gath[:],
                    out_offset=None,
                    in_=hm[:, :],
                    in_offset=bass.IndirectOffsetOnAxis(ap=rowidx[:, :1], axis=0),
                )
                # top8[:, 0] is the global max = the max of the gathered row, so
                # max_index finds its first in-row position directly.
                gidx8 = stats.tile([sz, 8], u32, tag=f"gidx8{off}")
                nc.vector.max_index(out=gidx8[:], in_max=top8[:], in_values=gath[:])

                nc.sync.dma_start(out=out_idx[off : off + sz, 0:1], in_=pwin8[:, 0:1])
                nc.sync.dma_start(out=out_idx[off : off + sz, 1:2], in_=gidx8[:, 0:1])

            # Stage A: stream all heatmap data once, per-partition max per image.
            groups = list(GROUPS)
            img = 0
            for i, g in enumerate(DMA_IMGS):
                t = load_pool.tile([P, g, FREE], f32, tag="hmtile")
                src = hm[img * P : (img + g) * P, :]
                src = src.rearrange("(g p) f -> p g f", g=g)
                eng = nc.sync if i % 2 == 0 else nc.scalar
                eng.dma_start(out=t[:], in_=src)
                nc.vector.reduce_max(redmax[:, img : img + g], t[:], axis=X)
                img += g
                if groups and img == groups[0][0] + groups[0][1]:
                    stage_bc(*groups.pop(0))
            assert not groups and img == TILES

    nc.compile()
    return nc


def _device_argmax(pred_heatmaps):
    """Run the 8-core SPMD kernel; return flat argmax per (b, k) as [B, K] int64."""
    from concourse.bass_utils import run_bass_kernel_spmd

    if "nc" not in _CACHE:
        _CACHE["nc"] = _build()
    nc = _CACHE["nc"]

    hm_all = np.ascontiguousarray(pred_heatmaps, dtype=np.float32).reshape(
        N_CORES, ROWS, FREE
    )
    in_maps = [{"hm": hm_all[c]} for c in range(N_CORES)]
    res = run_bass_kernel_spmd(
        nc,
        in_maps,
        core_ids=list(range(N_CORES)),
        **RUN_OPTS,
    )
    LAST_RESULTS["res"] = res
    idx = np.stack([r["out_idx"] for r in res.results], axis=0)  # [8, 32, 2] u32
    flat = idx[..., 0].astype(np.int64) * FREE + idx[..., 1].astype(np.int64)
    return flat.reshape(B, K)


def _host_loss(flat, gt_keypoints, ground_mask, naip_mask, worldcover_mask):
    """Evaluate the loss from flat argmax indices, mirroring reference float32 ops."""
    PADDING_LOSS_VALUE = np.float32(10.0)
    x_int = (flat % W).astype(np.float32)
    y_int = (flat // W).astype(np.float32)
    px = x_int / np.float32(W - 1)
    py = y_int / np.float32(H - 1)
    kp = np.stack([px, py], axis=-1)  # [B, K, 2] f32
    gt = np.asarray(gt_keypoints, dtype=np.float32).reshape(B, K, 2)
    loss_kpts = np.abs(kp - gt).sum(axis=(1, 2), dtype=np.float32)  # [B]

    def batch_mask_offset(mask):
        mask = np.asarray(mask, dtype=np.float32)
        Hm, Wm = mask.shape[1], mask.shape[2]
        kx = np.clip(kp[..., 0], np.float32(0.0), np.float32(Hm - 1))
        ky = np.clip(kp[..., 1], np.float32(0.0), np.float32(Wm - 1))
        ix = np.floor(kx).astype(np.int32)
        iy = np.floor(ky).astype(np.int32)
        clamped = np.stack([ix, iy], axis=-1).astype(np.float32)
        quant_off = np.abs(kp - clamped).sum(axis=(1, 2), dtype=np.float32)
        gathered = mask[np.arange(B)[:, None], ix, iy]  # [B, K]
        mask_off = ((np.float32(1.0) - gathered) * PADDING_LOSS_VALUE).sum(
            axis=1, dtype=np.float32
        )
        return quant_off + mask_off

    total = (
        loss_kpts
        + batch_mask_offset(ground_mask) * PADDING_LOSS_VALUE
        + batch_mask_offset(naip_mask) * PADDING_LOSS_VALUE
        + batch_mask_offset(worldcover_mask) * PADDING_LOSS_VALUE
    )
    return np.asarray(total.sum(dtype=np.float32), dtype=np.float32)


def kernel(
    pred_heatmaps,
    gt_keypoints,
    ground_padding_mask,
    naip_padding_mask,
    worldcover_padding_mask,
):
    pred_heatmaps = np.asarray(pred_heatmaps, dtype=np.float32)
    flat = _device_argmax(pred_heatmaps)
    return _host_loss(
        flat,
        gt_keypoints,
        ground_padding_mask,
        naip_padding_mask,
        worldcover_padding_mask,
    )


# revision 6
# speedup vs baseline: 1.1146x; 1.1146x over previous
# Trainium2 Bass kernel for nn_CustomKeypointLoss.
#
# reference(...) = sum over batch of:
#   sum_k |kp - gt|  +  10 * sum_{3 masks} [ quant_off + 10 * sum_k (1 - mask[b, ix, iy]) ]
# where kp = argmax-derived normalized keypoints from pred_heatmaps [B,K,512,512].
#
# Since kp in [0,1], ix=floor(kp_x) and iy=floor(kp_y) are in {0,1}: the masks are
# only read at [:, 0:2, 0:2].  All heavy lifting is the argmax over the 268MB of
# heatmaps.  Data-parallel over 8 cores (4 batch images each).
#
# Per-core device kernel:
#   view the core's heatmaps as hm[4096, 2048] (32 images x 128 chunks x 2048).
#   Stage A: 16x 2MB DMAs (two images per transfer, alternating between the two
#            HWDGE queues: sync + scalar) -> vector.reduce_max per image chunk ->
#            redmax[128, 32].  One full-data DVE scan, hidden under the DMA stream.
#   Stage B (per half of 16 images, so half 0 overlaps half 1's streaming):
#            PE-transpose redmax half [128,16] -> [16,128]; vector.max / max_index
#            give each image's global max and the FIRST 2048-chunk containing it.
#   Stage C: indirect-DMA gather of the 16 winning rows hm[img*128 + p_win, :]
#            from HBM; vector.max_index (reusing stage-B top8 maxes) gives the
#            first in-row index of the max.
#   Output: out_idx[32, 2] = (p_win, in_idx); flat argmax = p_win*2048 + in_idx.
#
# Host: reconstruct (x, y) = (flat % 512, flat // 512) and evaluate the (tiny)
# loss arithmetic in float32 exactly like the reference; sum partials over cores.

import numpy as np

B, K, H, W = 32, 8, 512, 512
N_CORES = 8
B_PER = B // N_CORES          # images per core
TILES = B_PER * K             # 32 heatmaps per core
P = 128                       # SBUF partitions
FREE = (H * W) // P           # 2048 elements per partition-row
ROWS = TILES * P              # 4096 rows in the per-core [ROWS, FREE] view
# Stream DMA sizes in images (1MB each): tapered so the pipeline starts and
# drains faster; 2MB in the middle for queue efficiency.
DMA_IMGS = [1, 1, 1, 1] + [2] * 12 + [1, 1, 1, 1]
assert sum(DMA_IMGS) == TILES
# Stage-B/C groups (image offset, count): group ends must align with DMA ends.
GROUPS = [(0, 16), (16, 8), (24, 8)]

_CACHE = {}
RUN_OPTS = {}  # test harness may set {"trace": True, ...}; harmless otherwise
LAST_RESULTS = {}  # test harness reads exec_time_ns from here


def _build():
    import concourse.bacc as bacc
    import concourse.tile as tile
    import concourse.mybir as mybir
    from concourse import bass
    from concourse.masks import make_identity

    f32 = mybir.dt.float32
    u32 = mybir.dt.uint32
    X = mybir.AxisListType.X

    nc = bacc.Bacc(
        "TRN2", target_bir_lowering=False, debug=False, enable_asserts=False
    )
    hm = nc.dram_tensor("hm", [ROWS, FREE], f32, kind="ExternalInput").ap()
    out_idx = nc.dram_tensor("out_idx", [TILES, 2], u32, kind="ExternalOutput").ap()

    with tile.TileContext(nc) as tc:
        with (
            tc.tile_pool(name="load", bufs=8) as load_pool,
            tc.tile_pool(name="stats", bufs=1) as stats,
            tc.tile_pool(name="psum", bufs=2, space="PSUM") as psum,
        ):
            ident = stats.tile([P, P], f32)
            make_identity(nc, ident[:])

            redmax = stats.tile([P, TILES], f32)

            def stage_bc(off, sz):
                """Cross-partition argmax + winning-row gather for images
                [off, off+sz)."""
                rm_t_ps = psum.tile([sz, P], f32, space="PSUM", tag="rm_t_ps")
                nc.tensor.transpose(
                    out=rm_t_ps[:],
                    in_=redmax[:, off : off + sz],
                    identity=ident[:],
                )
                # NOTE: sync + scalar instruction streams must contain ONLY the
                # heatmap stream DMAs: anything else placed there waits on
                # stage-B inputs and stalls all later DMA issues on that queue.
                rm_t = stats.tile([sz, P], f32, tag=f"rm_t{off}")
                nc.vector.tensor_copy(rm_t[:], rm_t_ps[:])

                top8 = stats.tile([sz, 8], f32, tag=f"top8{off}")
                nc.vector.max(out=top8[:], in_=rm_t[:])
                pwin8 = stats.tile([sz, 8], u32, tag=f"pwin8{off}")
                nc.vector.max_index(out=pwin8[:], in_max=top8[:], in_values=rm_t[:])

                # global row to gather = (off + img_local)*128 + p_win
                rowidx = stats.tile([sz, 1], u32, tag=f"rowidx{off}")
                nc.gpsimd.iota(
                    rowidx[:], pattern=[[0, 1]], base=off * P, channel_multiplier=P
                )
                nc.gpsimd.tensor_tensor(
                    out=rowidx[:], in0=rowidx[:], in1=pwin8[:, 0:1],
                    op=mybir.AluOpType.add,
                )

                gath = stats.tile([sz, FREE], f32, tag=f"gath{off}")
                nc.gpsimd.indirect_dma_start(
                    out=gath[:],
                    out_offset=None,
                    in_=hm[:, :],
                    in_offset=bass.IndirectOffsetOnAxis(ap=rowidx[:, :1], axis=0),
                )
                # top8[:, 0] is the global max = the max of the gathered row, so
                # max_index finds its first in-row position directly.
                gidx8 = stats.tile([sz, 8], u32, tag=f"gidx8{off}")
                nc.vector.max_index(out=gidx8[:], in_max=top8[:], in_values=gath[:])

                nc.gpsimd.dma_start(
                    out=out_idx[off : off + sz, 0:1], in_=pwin8[:, 0:1]
                )
                nc.gpsimd.dma_start(
                    out=out_idx[off : off + sz, 1:2], in_=gidx8[:, 0:1]
                )

            # Stage A: stream all heatmap data once, per-partition max per image.
            groups = list(GROUPS)
            img = 0
            for i, g in enumerate(DMA_IMGS):
                t = load_pool.tile([P, g, FREE], f32, tag="hmtile")
                src = hm[img * P : (img + g) * P, :]
                src = src.rearrange("(g p) f -> p g f", g=g)
                eng = nc.sync if i % 2 == 0 else nc.scalar
                eng.dma_start(out=t[:], in_=src)
                nc.vector.reduce_max(redmax[:, img : img + g], t[:], axis=X)
                img += g
                if groups and img == groups[0][0] + groups[0][1]:
                    stage_bc(*groups.pop(0))
            assert not groups and img == TILES

    nc.compile()
    return nc


def _device_argmax(pred_heatmaps):
    """Run the 8-core SPMD kernel; return flat argmax per (b, k) as [B, K] int64."""
    from concourse.bass_utils import run_bass_kernel_spmd

    if "nc" not in _CACHE:
        _CACHE["nc"] = _build()
    nc = _CACHE["nc"]

    hm_all = np.ascontiguousarray(pred_heatmaps, dtype=np.float32).reshape(
        N_CORES, ROWS, FREE
    )
    in_maps = [{"hm": hm_all[c]} for c in range(N_CORES)]
    res = run_bass_kernel_spmd(
        nc,
        in_maps,
        core_ids=list(range(N_CORES)),
        **RUN_OPTS,
    )
    LAST_RESULTS["res"] = res
    idx = np.stack([r["out_idx"] for r in res.results], axis=0)  # [8, 32, 2] u32
    flat = idx[..., 0].astype(np.int64) * FREE + idx[..., 1].astype(np.int64)
    return flat.reshape(B, K)


def _host_loss(flat, gt_keypoints, ground_mask, naip_mask, worldcover_mask):
    """Evaluate the loss from flat argmax indices, mirroring reference float32 ops."""
    PADDING_LOSS_VALUE = np.float32(10.0)
    x_int = (flat % W).astype(np.float32)
    y_int = (flat // W).astype(np.float32)
    px = x_int / np.float32(W - 1)
    py = y_int / np.float32(H - 1)
    kp = np.stack([px, py], axis=-1)  # [B, K, 2] f32
    gt = np.asarray(gt_keypoints, dtype=np.float32).reshape(B, K, 2)
    loss_kpts = np.abs(kp - gt).sum(axis=(1, 2), dtype=np.float32)  # [B]

    def batch_mask_offset(mask):
        mask = np.asarray(mask, dtype=np.float32)
        Hm, Wm = mask.shape[1], mask.shape[2]
        kx = np.clip(kp[..., 0], np.float32(0.0), np.float32(Hm - 1))
        ky = np.clip(kp[..., 1], np.float32(0.0), np.float32(Wm - 1))
        ix = np.floor(kx).astype(np.int32)
        iy = np.floor(ky).astype(np.int32)
        clamped = np.stack([ix, iy], axis=-1).astype(np.float32)
        quant_off = np.abs(kp - clamped).sum(axis=(1, 2), dtype=np.float32)
        gathered = mask[np.arange(B)[:, None], ix, iy]  # [B, K]
        mask_off = ((np.float32(1.0) - gathered) * PADDING_LOSS_VALUE).sum(
            axis=1, dtype=np.float32
        )
        return quant_off + mask_off

    total = (
        loss_kpts
        + batch_mask_offset(ground_mask) * PADDING_LOSS_VALUE
        + batch_mask_offset(naip_mask) * PADDING_LOSS_VALUE
        + batch_mask_offset(worldcover_mask) * PADDING_LOSS_VALUE
    )
    return np.asarray(total.sum(dtype=np.float32), dtype=np.float32)


def kernel(
    pred_heatmaps,
    gt_keypoints,
    ground_padding_mask,
    naip_padding_mask,
    worldcover_padding_mask,
):
    pred_heatmaps = np.asarray(pred_heatmaps, dtype=np.float32)
    flat = _device_argmax(pred_heatmaps)
    return _host_loss(
        flat,
        gt_keypoints,
        ground_padding_mask,
        naip_padding_mask,
        worldcover_padding_mask,
    )
